# revision 18
# baseline (speedup 1.0000x reference)
"""Trainium2 Bass kernel for the CgpHmm scaled-forward log-likelihood.

Strategy (hardcoded for inputs [32,10000,126], A [132,132], B [132,126], I0 [132]):
  - Each sequence's recursion is split at t=5000: a forward chain
    alpha_t = e_t * (A^T alpha_{t-1}) and a backward chain
    delta_t = e_t * (A delta_{t+1}).  Both have the identical compute shape
    z <- e * (L^T z) with L = A (fwd) or A^T (bwd), so a single SPMD program
    runs on all 8 cores: cores 0-3 forward (4 x 8 sequences), cores 4-7
    backward with the time axis pre-reversed on the host.
  - State-major layout [132 = 128+4 partitions, 8 seqs].  Four matmuls per
    step (128/4 block split of L), bf16 stationaries, fused u/v PSUM bank.
  - Emissions are computed on-device from the one-hot inputs (bf16, exact)
    via DMA-transpose + matmul against B^T — this is the memory-bound part.
  - Normalization every 16 steps: ones-matmul broadcast sum, Ln on ScalarE,
    reciprocal+rescale on VectorE.  Host combines:
    loglik = ll_f + ll_b + log(alpha^T A delta)  (exact; rescale errors cancel).
"""

import numpy as np
import ml_dtypes

bf16 = ml_dtypes.bfloat16

BATCH, T, AB = 32, 10000, 126
NS, NU, NV = 132, 128, 4
ABP = 128          # padded alphabet
B_SEQ = 8          # sequences per core
NCORE = 8
TC = 512           # time chunk
NCH = 10           # chunks per half  (TH = 5120 >= 5000)
S_STEPS = 4999     # chain steps (plus init at tau=0)
NORM = 14          # steps between norm measurements
FIRST_NORM = 8     # first measurement
NORM_DEFER = 2     # rescale applied this many steps after measurement
NB2 = 2 * B_SEQ    # fused u|v free width
ZPS_BUFS = 2       # chain psum rotation depth
SPLIT_EMUL = False # split the chain e-multiply into u/v DVE ops

# Pre-scale for Ln: c is ~126^-16 at each norm point, far outside the ACT
# Ln LUT's accurate domain.  Ln(c * LN_SCALE) keeps the input near 1; the
# host subtracts n_norms * log(LN_SCALE) afterwards.
LN_SCALE = np.float32(126.0) ** 16


def _build_nc(steps, nch, tc):
    import concourse.bacc as bacc
    import concourse.tile as tile
    from concourse import mybir

    f32 = mybir.dt.float32
    b16 = mybir.dt.bfloat16
    Ln = mybir.ActivationFunctionType.Ln

    th = nch * tc
    nc = bacc.Bacc("TRN2", target_bir_lowering=False, debug=False,
                   num_devices=NCORE)

    x = nc.dram_tensor("x", [B_SEQ, th, ABP], b16, kind="ExternalInput")
    w11 = nc.dram_tensor("w11", [NU, NU], b16, kind="ExternalInput")
    w21 = nc.dram_tensor("w21", [NV, NU], b16, kind="ExternalInput")
    w12 = nc.dram_tensor("w12", [NU, NV], b16, kind="ExternalInput")
    w22 = nc.dram_tensor("w22", [NV, NV], b16, kind="ExternalInput")
    bt1 = nc.dram_tensor("bt1", [ABP, NU], b16, kind="ExternalInput")
    bt2 = nc.dram_tensor("bt2", [ABP, NV], b16, kind="ExternalInput")
    q = nc.dram_tensor("q", [NS, 1], f32, kind="ExternalInput")
    out = nc.dram_tensor("out", [3, NU, B_SEQ], f32, kind="ExternalOutput")

    with tile.TileContext(nc) as tcx:
        with (
            tcx.tile_pool(name="const", bufs=1) as const,
            tcx.tile_pool(name="xt", bufs=3) as xtp,
            tcx.tile_pool(name="eps", bufs=2, space="PSUM") as epsp,
            tcx.tile_pool(name="epsv", bufs=max(1, 5 - ZPS_BUFS),
                          space="PSUM") as epsvp,
            tcx.tile_pool(name="zps", bufs=ZPS_BUFS, space="PSUM") as zpsp,
            tcx.tile_pool(name="cps", bufs=1, space="PSUM") as cpsp,
            tcx.tile_pool(name="z", bufs=4) as zp,
            tcx.tile_pool(name="small", bufs=4) as smallp,
            tcx.tile_pool(name="eil", bufs=1) as eilp,
        ):
            # ---- constants into SBUF ----
            w11s = const.tile([NU, NU], b16)
            w21s = const.tile([NV, NU], b16)
            w12s = const.tile([NU, NV], b16)
            w22s = const.tile([NV, NV], b16)
            bt1s = const.tile([ABP, NU], b16)
            bt2s = const.tile([ABP, NV], b16)
            qu = const.tile([NU, 1], f32)
            qv = const.tile([NV, 1], f32)
            for dst, src in ((w11s, w11), (w21s, w21), (w12s, w12),
                             (w22s, w22), (bt1s, bt1), (bt2s, bt2)):
                nc.sync.dma_start(out=dst[:], in_=src[:])
            nc.sync.dma_start(out=qu[:], in_=q[0:NU, :])
            nc.sync.dma_start(out=qv[:], in_=q[NU:NS, :])

            ones_u = const.tile([NU, NU], b16)
            ones_v = const.tile([NV, NU], b16)
            nc.vector.memset(ones_u[:], 1.0)
            nc.vector.memset(ones_v[:], 1.0)

            ll_acc = const.tile([1, B_SEQ], f32)
            nc.vector.memset(ll_acc[:], 0.0)

            # emission buffers (ping/pong), zero once so pad regions stay 0
            e_il0 = eilp.tile([NU, tc * NB2], f32, tag="eil0")
            e_il1 = eilp.tile([NU, tc * NB2], f32, tag="eil1")
            e_il = [e_il0, e_il1]
            nc.gpsimd.memset(e_il[0][:], 0.0)
            nc.gpsimd.memset(e_il[1][:], 0.0)

            # chain psum tiles (ping/pong), zero once so pad region stays 0
            zps = []
            for _zi in range(ZPS_BUFS):
                _zt = zpsp.tile([NU, NB2], f32, tag="zps", name=f"zps{_zi}")
                nc.vector.memset(_zt[:], 0.0)
                zps.append(_zt)

            cps = cpsp.tile([NU, B_SEQ], f32)

            # ---- emission chunk production ----
            # one unit = one sequence's slice of a chunk; units are emitted
            # interleaved with chain steps so the big N=512 matmuls and ACT
            # copies never burst ahead of chain matmuls in the engine FIFOs.
            def emit_unit(ci, s):
                buf = e_il[ci % 2]
                bufv = buf.rearrange("p (t k) -> p t k", k=NB2)
                xt = xtp.tile([ABP, tc], b16, tag="xt", name=f"xt_{ci}_{s}")
                nc.sync.dma_start_transpose(
                    out=xt[:], in_=x[s, ci * tc:(ci + 1) * tc, :])
                pu = epsp.tile([NU, tc], f32, tag="eps", name=f"pu_{ci}_{s}")
                nc.tensor.matmul(pu[:], lhsT=bt1s[:], rhs=xt[:],
                                 start=True, stop=True)
                pv = epsvp.tile([NV, tc], f32, tag="epsv", name=f"pv_{ci}_{s}")
                nc.tensor.matmul(pv[:], lhsT=bt2s[:], rhs=xt[:],
                                 start=True, stop=True)
                nc.scalar.copy(out=bufv[:, :, s], in_=pu[:])
                nc.scalar.copy(out=bufv[0:NV, :, B_SEQ + s], in_=pv[:])

            def produce_chunk(ci):
                for s in range(B_SEQ):
                    emit_unit(ci, s)

            produce_chunk(0)
            if nch > 1:
                produce_chunk(1)
            pending = []
            unit_every = max(1, tc // B_SEQ)

            # ---- init: z0 = q * e_0 ----
            buf0 = e_il[0]
            z0 = zp.tile([NU, NB2], b16, tag="z")
            nc.vector.tensor_scalar_mul(z0[:, 0:B_SEQ], buf0[:, 0:B_SEQ], qu[:])
            nc.vector.tensor_scalar_mul(z0[0:NV, B_SEQ:NB2],
                                        buf0[0:NV, B_SEQ:NB2], qv[:])
            zu, zv = z0[:, 0:B_SEQ], z0[0:NV, B_SEQ:NB2]

            # ---- main chain ----
            for tau in range(1, steps + 1):
                ci, t = divmod(tau, tc)
                if t == 0 and ci + 1 < nch:
                    pending.extend((ci + 1, s) for s in range(B_SEQ))
                if pending and tau % unit_every == unit_every // 2:
                    emit_unit(*pending.pop(0))
                buf = e_il[ci % 2]
                zpt = zps[tau % ZPS_BUFS]
                nc.tensor.matmul(zpt[:, 0:B_SEQ], lhsT=w11s[:], rhs=zu,
                                 start=True, stop=False)
                nc.tensor.matmul(zpt[:, 0:B_SEQ], lhsT=w21s[:], rhs=zv,
                                 start=False, stop=True)
                nc.tensor.matmul(zpt[0:NV, B_SEQ:NB2], lhsT=w12s[:], rhs=zu,
                                 start=True, stop=False)
                nc.tensor.matmul(zpt[0:NV, B_SEQ:NB2], lhsT=w22s[:], rhs=zv,
                                 start=False, stop=True)
                z = zp.tile([NU, NB2], b16, tag="z")
                if SPLIT_EMUL:
                    nc.vector.tensor_mul(z[:, 0:B_SEQ], zpt[:, 0:B_SEQ],
                                         buf[:, t * NB2:t * NB2 + B_SEQ])
                    nc.vector.tensor_mul(z[0:NV, B_SEQ:NB2],
                                         zpt[0:NV, B_SEQ:NB2],
                                         buf[0:NV, t * NB2 + B_SEQ:(t + 1) * NB2])
                else:
                    nc.vector.tensor_mul(z[:], zpt[:],
                                         buf[:, t * NB2:(t + 1) * NB2])
                zu, zv = z[:, 0:B_SEQ], z[0:NV, B_SEQ:NB2]

                # measurement: sum z, log to ll, reciprocal -> rc (deferred)
                if (tau - FIRST_NORM) % NORM == 0 and \
                        FIRST_NORM <= tau <= steps - NORM_DEFER:
                    nc.tensor.matmul(cps[:], lhsT=ones_u[:], rhs=zu,
                                     start=True, stop=False)
                    nc.tensor.matmul(cps[:], lhsT=ones_v[:], rhs=zv,
                                     start=False, stop=True)
                    llt = smallp.tile([1, B_SEQ], f32, tag="llt")
                    nc.scalar.activation(llt[:], cps[0:1, :], Ln,
                                         scale=float(LN_SCALE))
                    nc.vector.tensor_add(ll_acc[:], ll_acc[:], llt[:])
                    rc = smallp.tile([NU, B_SEQ], b16, tag="rc")
                    # rc precision is self-correcting: the rescale that z
                    # actually receives is folded back in via the final
                    # host-side dot product; only log(c) enters ll.
                    with nc.allow_low_precision(reason="self-correcting rescale"):
                        nc.vector.reciprocal(rc[:], cps[:])
                    pending_rc = rc
                # application: two steps later, rc is ready -> cheap rescale
                if (tau - FIRST_NORM - NORM_DEFER) % NORM == 0 and \
                        tau >= FIRST_NORM + NORM_DEFER:
                    zn = zp.tile([NU, NB2], b16, tag="z")
                    nc.vector.tensor_mul(zn[:, 0:B_SEQ], zu, pending_rc[:])
                    nc.vector.tensor_mul(zn[0:NV, B_SEQ:NB2], zv,
                                         pending_rc[0:NV, :])
                    zu, zv = zn[:, 0:B_SEQ], zn[0:NV, B_SEQ:NB2]

            # ---- outputs ----
            zout = smallp.tile([NU, NB2], f32, tag="zout")
            nc.vector.tensor_copy(zout[:, 0:B_SEQ], zu)
            nc.vector.tensor_copy(zout[0:NV, B_SEQ:NB2], zv)
            nc.sync.dma_start(out=out[0, :, :], in_=zout[:, 0:B_SEQ])
            nc.sync.dma_start(out=out[1, 0:NV, :], in_=zout[0:NV, B_SEQ:NB2])
            nc.sync.dma_start(out=out[2, 0:1, :], in_=ll_acc[:])

    nc.compile()
    return nc


def _host_prep(inputs, A, B, I0, steps=S_STEPS, nch=NCH, tc=TC):
    """Build the 8 per-core input maps."""
    th = nch * tc
    half = steps + 1
    X = np.ascontiguousarray(inputs).astype(bf16)  # exact for one-hot

    A32 = np.asarray(A, np.float32)
    B32 = np.asarray(B, np.float32)
    I032 = np.asarray(I0, np.float32)

    Bt = np.zeros((ABP, NS), np.float32)
    Bt[:AB, :] = B32.T
    bt1 = Bt[:, 0:NU].astype(bf16)
    bt2 = Bt[:, NU:NS].astype(bf16)

    def wtiles(L):
        L = L.astype(bf16)
        return {
            "w11": np.ascontiguousarray(L[0:NU, 0:NU]),
            "w21": np.ascontiguousarray(L[NU:NS, 0:NU]),
            "w12": np.ascontiguousarray(L[0:NU, NU:NS]),
            "w22": np.ascontiguousarray(L[NU:NS, NU:NS]),
        }

    wf = wtiles(A32)        # fwd: lhsT = A
    wb = wtiles(A32.T)      # bwd: lhsT = A^T
    qf = I032.reshape(NS, 1)
    qb = np.ones((NS, 1), np.float32)

    in_maps = []
    for c in range(NCORE):
        fwd = c < 4
        g = c % 4
        seqs = slice(g * B_SEQ, (g + 1) * B_SEQ)
        xs = np.zeros((B_SEQ, th, ABP), bf16)
        if fwd:
            xs[:, :half, :AB] = X[seqs, :half, :]
        else:
            # time-reversed: rows t = T-1 .. T-half
            xs[:, :half, :AB] = X[seqs, : T - half - 1: -1, :]
        m = {"x": xs, "bt1": bt1, "bt2": bt2,
             "q": qf if fwd else qb}
        m.update(wf if fwd else wb)
        in_maps.append(m)
    return in_maps


def _host_combine(results, A, steps=S_STEPS):
    A64 = np.asarray(A, np.float64)
    n_norms = sum(1 for tau in range(1, steps + 1)
                  if (tau - FIRST_NORM) % NORM == 0
                  and FIRST_NORM <= tau <= steps - NORM_DEFER)
    ln_corr = n_norms * np.log(np.float64(LN_SCALE))
    loglik = np.zeros(BATCH, np.float32)
    for g in range(4):
        of = results[g]["out"]
        ob = results[g + 4]["out"]
        for k in range(B_SEQ):
            alpha = np.concatenate([of[0, :, k], of[1, 0:NV, k]]).astype(np.float64)
            delta = np.concatenate([ob[0, :, k], ob[1, 0:NV, k]]).astype(np.float64)
            dot = alpha @ A64 @ delta
            loglik[g * B_SEQ + k] = (np.float64(of[2, 0, k]) - ln_corr +
                                     np.float64(ob[2, 0, k]) - ln_corr +
                                     np.log(dot))
    return loglik


_NC_CACHE = {}


def _get_nc(steps=S_STEPS, nch=NCH, tc=TC):
    key = (steps, nch, tc)
    if key not in _NC_CACHE:
        _NC_CACHE[key] = _build_nc(steps, nch, tc)
    return _NC_CACHE[key]


def kernel(inputs, A, B, I0, trace=False):
    from concourse.bass_utils import run_bass_kernel_spmd

    nc = _get_nc()
    in_maps = _host_prep(inputs, A, B, I0)
    res = run_bass_kernel_spmd(nc, in_maps, list(range(NCORE)), trace=trace)
    out = _host_combine(res.results, A)
    if trace:
        return out, res
    return out
